# revision 26
# baseline (speedup 1.0000x reference)
"""Trainium2 Bass kernel for BaseAttnPredictNet (pre-LN MHA with zero-attn
slot, gated output combination, residual).

v2 strategy:
- Host compacts masked-out keys (mask=0 keys contribute exactly 0 to
  softmax) and deals query rows so each core gets ~equal unmasked rows,
  placed first in its 512-row block. Attention runs only on QA (~288)
  query columns and KC (~1152) compacted keys instead of 512x2176.
- Flipped PV: av[dh+den, q] accumulated in PSUM over key blocks with a
  valid-indicator column producing the softmax denominator; normalize by
  rank-1 PE broadcast of qm/den.
- Natural-layout gate / output projection / combine: activations serve as
  matmul lhsT so no transposes back from feature-major space are needed;
  gate bias enters via a ones-row rank-1 matmul.
- QK head pairs run as row-tiled (base partition 0/64) matmul pairs.
- Host pre-casts weights and k/v to bf16 (halves DMA).
- Sharding: cores 0-3 batch 0, cores 4-7 batch 1; host inverse-permutes
  row order on gather.
"""

import numpy as np
import ml_dtypes

import concourse.bass as bass
import concourse.bacc as bacc
import concourse.mybir as mybir
import concourse.tile as tile
from concourse.bass_utils import run_bass_kernel_spmd
from concourse.masks import make_identity

B, Q, KLEN, D = 2, 2048, 2048, 512
H, DH = 8, 64
P = 128
QS = 512          # query rows per core (output responsibility)
ND = D // P       # 4 feature blocks
NG = 2 * D // P
NCORES = 8
SCALE = 0.125
LN_EPS = 1e-5

F32 = mybir.dt.float32
BF16 = mybir.dt.bfloat16
AF = mybir.ActivationFunctionType
OP = mybir.AluOpType
BF = ml_dtypes.bfloat16


def _build(QA: int, KC: int) -> bass.Bass:
    NKC = KC // P
    nc = bacc.Bacc("TRN2", target_bir_lowering=False, debug=False)

    din = {}
    for name, shape, dt in (
        ("qf", [QS, D], BF16),
        ("kc", [KC, D], BF16),
        ("vc", [KC, D], BF16),
        ("wq", [D, D], BF16),
        ("wk", [D, D], BF16),
        ("wv", [D, D], BF16),
        ("wo", [D, D], BF16),
        ("wop", [D, D], BF16),
        ("gwq", [D, D], BF16),
        ("gbn", [1, D], BF16),
        ("kval", [P, NKC], F32),
        ("qm", [1, QS], F32),
    ):
        din[name] = nc.dram_tensor(name, shape, dt, kind="ExternalInput")
    out_d = nc.dram_tensor("out", [QS, D], BF16, kind="ExternalOutput")

    with tile.TileContext(nc) as tc:
        _body(nc, tc, din, out_d, QA, KC)
    nc.compile()
    return nc


def _body(nc, tc, din, out_d, QA, KC):
    NKC = KC // P
    from contextlib import ExitStack

    ctx = ExitStack()
    with ctx:
        persist = ctx.enter_context(tc.tile_pool(name="persist", bufs=1))
        stats = ctx.enter_context(tc.tile_pool(name="stats", bufs=6))

        ident_bf = persist.tile([P, P], BF16)
        make_identity(nc, ident_bf)
        ones_bf = persist.tile([P, P], BF16)
        nc.vector.memset(ones_bf, 1.0)
        eps_t = persist.tile([P, 1], F32)
        nc.vector.memset(eps_t, LN_EPS)

        kval = persist.tile([P, NKC], F32)
        gbn = persist.tile([1, D], BF16)
        qm_bc = persist.tile([P, QS], F32)

        # ---- persistent activations ----
        knT = persist.tile([P, ND, KC], BF16)
        vh_aug = persist.tile([P, NKC, H, DH + 1], BF16)
        qnT = persist.tile([P, ND, QA], BF16)
        qT = persist.tile([P, ND, QS], BF16)
        qhT = [persist.tile([P, QA], BF16, name=f"qhT{a}") for a in range(ND)]
        khT = [persist.tile([P, KC], BF16, name=f"khT{a}") for a in range(ND)]
        avT = persist.tile([P, ND, QA], BF16)
        po = persist.tile([P, ND, D], BF16)
        g_nat = persist.tile([P, ND, D], BF16)
        outn = persist.tile([P, ND, D], BF16)
        qf_s = persist.tile([P, ND, D], BF16)

        # PSUM budget is 8 banks; transpose pool (2) + projection pool (4)
        # coexist, both released (LIFO) before the attention pools (4+3+1).
        ptp = tc.alloc_tile_pool(name="ptp", bufs=2, space="PSUM")
        pp = tc.alloc_tile_pool(name="pp", bufs=4, space="PSUM")

        # natural-layout staging for k/v (released after use)
        vcp = tc.alloc_tile_pool(name="vcp", bufs=1)
        vc_s = vcp.tile([P, NKC, D], BF16)
        kcp = tc.alloc_tile_pool(name="kcp", bufs=1)
        kc_s = kcp.tile([P, NKC, D], BF16)

        # chunked input DMAs (3 blocks each => ~384KB)
        for c0 in range(0, NKC, 3):
            cw = min(3, NKC - c0)
            src = din["kc"][c0 * P : (c0 + cw) * P, :].rearrange("(c p) d -> p c d", p=P)
            nc.sync.dma_start(out=kc_s[:, c0 : c0 + cw, :], in_=src)
        for c0 in range(0, NKC, 3):
            cw = min(3, NKC - c0)
            src = din["vc"][c0 * P : (c0 + cw) * P, :].rearrange("(c p) d -> p c d", p=P)
            nc.sync.dma_start(out=vc_s[:, c0 : c0 + cw, :], in_=src)
        nc.sync.dma_start(
            out=qf_s, in_=din["qf"][:, :].rearrange("(a p) d -> p a d", p=P)
        )

        nc.sync.dma_start(out=kval, in_=din["kval"][:, :])
        nc.sync.dma_start(out=gbn, in_=din["gbn"][:, :])
        _qm_ap = din["qm"][:, :]
        nc.sync.dma_start(
            out=qm_bc,
            in_=bass.AP(tensor=_qm_ap.tensor, offset=_qm_ap.offset, ap=[[0, P], [1, QS]]),
        )

        # weights after activations: k/v/q feed the LN pipeline first
        w_s = {}
        for wname in ("wk", "wv", "wq", "wo", "wop", "gwq"):
            wt = persist.tile([P, ND, D], BF16, name=f"{wname}_s")
            nc.sync.dma_start(
                out=wt, in_=din[wname][:, :].rearrange("(b p) d -> p b d", p=P)
            )
            w_s[wname] = wt

        def ln_stats2(xtiles):
            """LN (bias,scale) for up to 2 [rows,D] tiles; batches the sqrt."""
            cw = len(xtiles)
            mv = stats.tile([P, 2, 2], F32, name="bnagg")
            for cc, (xt, rows) in enumerate(xtiles):
                st = stats.tile([P, 6], F32, name="bnst")
                nc.vector.bn_stats(out=st[:rows], in_=xt)
                nc.vector.bn_aggr(out=mv[:rows, cc, :], in_=st[:rows])
            std = stats.tile([P, 2], F32, name="std")
            nc.scalar.activation(
                out=std[:, :cw], in_=mv[:, 0:cw, 1], func=AF.Sqrt, bias=eps_t
            )
            rstd = stats.tile([P, 2], F32, name="rstd")
            nc.vector.reciprocal(rstd[:, :cw], std[:, :cw])
            nm2 = stats.tile([P, 2], F32, name="nm2")
            nc.vector.tensor_tensor(
                out=nm2[:, :cw], in0=mv[:, 0:cw, 0], in1=rstd[:, :cw], op=OP.mult
            )
            nc.vector.tensor_scalar_mul(nm2[:, :cw], nm2[:, :cw], -1.0)
            return nm2, rstd

        def ln_transpose(src_tile, nrows, dstT, apply_eng, copy_engs):
            """Per-128-row-block LN + PE transpose into dstT [P, ND, nrows]."""
            nblk = (nrows + P - 1) // P
            ei = 0
            for c0 in range(0, nblk, 2):
                cw = min(2, nblk - c0)
                xt = []
                for cc in range(cw):
                    rows = min(P, nrows - (c0 + cc) * P)
                    xt.append((src_tile[:rows, c0 + cc, :], rows))
                nm2, rstd = ln_stats2(xt)
                for cc in range(cw):
                    c = c0 + cc
                    rows = xt[cc][1]
                    xn = stats.tile([P, D], BF16, name="xnorm")
                    if apply_eng == "gps":
                        nc.gpsimd.tensor_scalar(
                            out=xn[:rows],
                            in0=xt[cc][0],
                            scalar1=nm2[:rows, cc : cc + 1],
                            scalar2=rstd[:rows, cc : cc + 1],
                            op0=OP.add,
                            op1=OP.mult,
                        )
                    else:
                        nc.vector.tensor_scalar(
                            out=xn[:rows],
                            in0=xt[cc][0],
                            scalar1=nm2[:rows, cc : cc + 1],
                            scalar2=rstd[:rows, cc : cc + 1],
                            op0=OP.add,
                            op1=OP.mult,
                        )
                    pt = ptp.tile([P, ND, P], BF16, name="pt")
                    for b in range(ND):
                        nc.tensor.transpose(
                            pt[:, b, :rows],
                            xn[:rows, b * P : (b + 1) * P],
                            ident_bf[:rows, :rows],
                        )
                    eng = copy_engs[ei % len(copy_engs)]
                    ei += 1
                    if eng == "act":
                        nc.scalar.copy(dstT[:, :, c * P : c * P + rows], pt[:, :, :rows])
                    elif eng == "gps":
                        nc.gpsimd.tensor_copy(dstT[:, :, c * P : c * P + rows], pt[:, :, :rows])
                    else:
                        nc.vector.tensor_copy(dstT[:, :, c * P : c * P + rows], pt[:, :, :rows])

        # ---- k: LN + transpose, then k projection ----
        ln_transpose(kc_s, KC, knT, "vec", ("act", "vec"))

        # khT[a] = Wk'.T @ knT   (j in chunks of KC/3)
        JC = KC // 3
        for a in range(ND):
            for j0 in range(0, KC, JC):
                ps = pp.tile([P, 512], F32, name="pp_t")
                for b in range(ND):
                    nc.tensor.matmul(
                        ps[:, :JC],
                        w_s["wk"][:, b, a * P : (a + 1) * P],
                        knT[:, b, j0 : j0 + JC],
                        start=(b == 0),
                        stop=(b == ND - 1),
                    )
                nc.scalar.copy(khT[a][:, j0 : j0 + JC], ps[:, :JC])
        kcp.release()

        # ---- v: LN + transpose (into temp), then vh (natural, valid-scaled) ----
        vtp = tc.alloc_tile_pool(name="vtp", bufs=1)
        vnT = vtp.tile([P, ND, KC], BF16)
        ln_transpose(vc_s, KC, vnT, "vec", ("vec", "act"))
        for c in range(NKC):
            ps = pp.tile([P, 512], F32, name="pp_t")
            for b in range(ND):
                nc.tensor.matmul(
                    ps,
                    vnT[:, b, c * P : (c + 1) * P],
                    w_s["wv"][:, b, :],
                    start=(b == 0),
                    stop=(b == ND - 1),
                )
            pp3 = ps.rearrange("p (h e) -> p h e", h=H)
            nc.scalar.activation(
                out=vh_aug[:, c, :, 0:DH],
                in_=pp3,
                func=AF.Copy,
                scale=kval[:, c : c + 1],
            )
            nc.vector.tensor_copy(
                vh_aug[:, c, :, DH : DH + 1],
                kval[:, c : c + 1].unsqueeze(1).broadcast_to((P, H, 1)),
            )
        vtp.release()
        vcp.release()

        # ---- q: LN + transpose (first QA rows), raw bf16 transpose (all rows) ----
        ln_transpose(qf_s, QA, qnT, "vec", ("vec", "act"))
        for a in range(ND):
            pt = ptp.tile([P, ND, P], BF16, name="pt")
            for b in range(ND):
                nc.tensor.transpose(pt[:, b, :], qf_s[:, a, b * P : (b + 1) * P], ident_bf)
            nc.vector.tensor_copy(qT[:, :, a * P : (a + 1) * P], pt)

        # qhT[a] = Wq'.T @ qnT
        for a in range(ND):
            ps = pp.tile([P, 512], F32, name="pp_t")
            for b in range(ND):
                nc.tensor.matmul(
                    ps[:, :QA],
                    w_s["wq"][:, b, a * P : (a + 1) * P],
                    qnT[:, b, :],
                    start=(b == 0),
                    stop=(b == ND - 1),
                )
            nc.vector.tensor_copy(qhT[a], ps[:, :QA])

        # ---- attention: per head-pair, QK row-tiled pairs -> exp -> PV ----
        pp.release()
        ptp.release()
        expp = tc.alloc_tile_pool(name="expp", bufs=3)
        psS = tc.alloc_tile_pool(name="psS", bufs=1, space="PSUM")
        pav = tc.alloc_tile_pool(name="pav", bufs=5, space="PSUM")
        pfb = tc.alloc_tile_pool(name="pfb", bufs=1, space="PSUM")

        for nb in range(ND):
            av2 = [pav.tile([P, 512], F32, name="pav_t") for _ in range(2)]
            for c in range(NKC):
                ps2 = psS.tile([P, 2, 512], F32, name="pS2")
                for i in range(2):
                    r0 = i * DH
                    nc.tensor.matmul(
                        ps2[:, i, :QA],
                        khT[nb][r0 : r0 + DH, c * P : (c + 1) * P],
                        qhT[nb][r0 : r0 + DH, :],
                        start=True,
                        stop=True,
                    )
                e2 = expp.tile([P, 2, QA], BF16, name="expS")
                nc.scalar.activation(
                    out=e2, in_=ps2[:, :, :QA], func=AF.Exp, scale=SCALE
                )
                for i in range(2):
                    nc.tensor.matmul(
                        av2[i][0 : DH + 1, :QA],
                        vh_aug[:, c, 2 * nb + i, :],
                        e2[:, i, :],
                        start=(c == 0),
                        stop=(c == NKC - 1),
                    )
            # normalize: avT rows = [head even 0:64, head odd 64:128]
            for i in range(2):
                av = av2[i]
                ftf = stats.tile([P, QA], F32, name="ftf")
                nc.vector.reciprocal(ftf[DH : DH + 1, :], av[DH : DH + 1, :QA])
                ft = stats.tile([P, QA], BF16, name="ft")
                nc.vector.tensor_tensor(
                    out=ft[DH : DH + 1, :],
                    in0=ftf[DH : DH + 1, :],
                    in1=qm_bc[DH : DH + 1, :QA],
                    op=OP.mult,
                )
                fb = pfb.tile([P, 512], F32, name="fb_t")
                nc.tensor.matmul(
                    fb[0:DH, :QA],
                    ones_bf[DH : DH + 1, 0:DH],
                    ft[DH : DH + 1, :],
                    start=True,
                    stop=True,
                )
                fbs = stats.tile([P, QA], BF16, name="fbs")
                nc.vector.tensor_copy(fbs[0:DH, :], fb[0:DH, :QA])
                if i == 0:
                    nc.vector.tensor_tensor(
                        out=avT[0:DH, nb, :],
                        in0=av[0:DH, :QA],
                        in1=fbs[0:DH, :],
                        op=OP.mult,
                    )
                else:
                    avtmp = stats.tile([P, QA], BF16, name="avtmp")
                    nc.vector.tensor_tensor(
                        out=avtmp[0:DH, :],
                        in0=av[0:DH, :QA],
                        in1=fbs[0:DH, :],
                        op=OP.mult,
                    )
                    sh = pav.tile([P, 512], F32, name="pav_t")
                    nc.tensor.matmul(
                        sh[DH : 2 * DH, :QA],
                        ident_bf[0:DH, 0:DH],
                        avtmp[0:DH, :],
                        start=True,
                        stop=True,
                    )
                    nc.vector.tensor_copy(avT[DH:P, nb, :], sh[DH : 2 * DH, :QA])

        # ---- output projection (natural): po[qblk] = avT.T @ Wo ----
        pfb.release()
        pav.release()
        psS.release()
        expp.release()
        pog = tc.alloc_tile_pool(name="pog", bufs=4, space="PSUM")
        NQB = (QA + P - 1) // P  # q blocks overlapping attention rows
        nc.gpsimd.memset(po[:, :, :], 0.0)
        for a in range(NQB):
            rows = min(P, QA - a * P)
            ps = pog.tile([P, 512], F32, name="pog_t")
            for b in range(ND):
                nc.tensor.matmul(
                    ps[:rows, :],
                    avT[:, b, a * P : a * P + rows],
                    w_s["wo"][:, b, :],
                    start=(b == 0),
                    stop=(b == ND - 1),
                )
            nc.vector.tensor_copy(po[:rows, a, :], ps[:rows, :])

        # ---- gate (natural): g = sigmoid(q@gwq + av_n@wop + gb) ----
        for a in range(ND):
            ps = pog.tile([P, 512], F32, name="pog_t")
            nc.tensor.matmul(
                ps, ones_bf[0:1, 0:P], gbn[0:1, :], start=True, stop=False
            )
            rows = min(max(QA - a * P, 0), P)
            for b in range(ND):
                nc.tensor.matmul(
                    ps,
                    qT[:, b, a * P : (a + 1) * P],
                    w_s["gwq"][:, b, :],
                    start=False,
                    stop=(b == ND - 1 and rows == 0),
                )
            if rows > 0:
                for b in range(ND):
                    nc.tensor.matmul(
                        ps[:rows, :],
                        avT[:, b, a * P : a * P + rows],
                        w_s["wop"][:, b, :],
                        start=False,
                        stop=(b == ND - 1),
                    )
            nc.scalar.activation(out=g_nat[:, a, :], in_=ps, func=AF.Sigmoid)

        # ---- combine: out = (q + po) + g*(q - po) ----
        for a in range(ND):
            s = stats.tile([P, D], BF16, name="fin_s")
            nc.vector.tensor_tensor(
                out=s, in0=qf_s[:, a, :], in1=po[:, a, :], op=OP.subtract
            )
            m = stats.tile([P, D], BF16, name="fin_m")
            nc.vector.tensor_tensor(out=m, in0=g_nat[:, a, :], in1=s, op=OP.mult)
            r = stats.tile([P, D], BF16, name="fin_r")
            nc.vector.tensor_tensor(
                out=r, in0=qf_s[:, a, :], in1=po[:, a, :], op=OP.add
            )
            nc.vector.tensor_tensor(out=outn[:, a, :], in0=m, in1=r, op=OP.add)

        dst = out_d[:, :].rearrange("(a p) d -> p a d", p=P)
        nc.sync.dma_start(out=dst, in_=outn)
        pog.release()


_CACHE: dict = {}


def _ceil(x, g):
    return -(-x // g) * g


def make_in_maps(inputs):
    q = np.asarray(inputs["query"], np.float32)
    k = np.asarray(inputs["key"], np.float32)
    v = np.asarray(inputs["value"], np.float32)
    wq = np.asarray(inputs["weight_q"], np.float32)
    wk = np.asarray(inputs["weight_k"], np.float32)
    wv = np.asarray(inputs["weight_v"], np.float32)
    wo = np.asarray(inputs["weight_o"], np.float32)
    gw = np.asarray(inputs["g_w"], np.float32)
    gb = np.asarray(inputs["g_b"], np.float32)
    qmask = np.asarray(inputs["query_mask"])
    kmask = np.asarray(inputs["key_mask"])
    gams = [np.asarray(inputs[n], np.float32) for n in ("q_gamma", "k_gamma", "v_gamma")]
    bets = [np.asarray(inputs[n], np.float32) for n in ("q_beta", "k_beta", "v_beta")]
    assert all(np.all(bt == 0.0) for bt in bets), "beta path not implemented"

    # fold LN gamma into projection weight rows (free on host)
    wqf = wq * gams[0][:, None]
    wkf = wk * gams[1][:, None]
    wvf = wv * gams[2][:, None]
    wopf = wo @ gw[D:, :]
    gwqf = gw[:D, :]

    per_batch = NCORES // B

    # --- key compaction (+ zero-attn slot) ---
    kidx = [np.where(kmask[b] != 0)[0] for b in range(B)]
    KC = _ceil(max(len(ix) + 1 for ix in kidx), P)
    NKC = KC // P
    kcs, vcs, kvals = [], [], []
    for b in range(B):
        ix = kidx[b]
        n = len(ix)
        kc = np.zeros((KC, D), np.float32)
        vc = np.zeros((KC, D), np.float32)
        kc[:n] = k[b][ix]
        vc[:n] = v[b][ix]
        kvc = np.zeros(KC, np.float32)
        kvc[: n + 1] = 1.0  # compacted keys + zero-attn slot
        kcs.append(kc.astype(BF))
        vcs.append(vc.astype(BF))
        kvals.append(np.ascontiguousarray(kvc.reshape(NKC, P).T))

    # --- query deal: each core gets its batch's unmasked rows c::4 first ---
    rows_per_core = []
    na_per_core = []
    for b in range(B):
        un = np.where(qmask[b] != 0)[0]
        ma = np.where(qmask[b] == 0)[0]
        parts = [list(un[c::per_batch]) for c in range(per_batch)]
        mi = 0
        for c in range(per_batch):
            need = QS - len(parts[c])
            parts[c] = parts[c] + list(ma[mi : mi + need])
            mi += need
        assert mi == len(ma)
        for c in range(per_batch):
            rows_per_core.append(np.array(parts[c], np.int64))
            na_per_core.append(int((qmask[b][parts[c]] != 0).sum()))
    QA = max(_ceil(max(na_per_core), 32), 32)

    wmaps = {
        "wq": np.ascontiguousarray(wqf.astype(BF)),
        "wk": np.ascontiguousarray(wkf.astype(BF)),
        "wv": np.ascontiguousarray(wvf.astype(BF)),
        "wo": np.ascontiguousarray(wo.astype(BF)),
        "wop": np.ascontiguousarray(wopf.astype(BF)),
        "gwq": np.ascontiguousarray(gwqf.astype(BF)),
        "gbn": np.ascontiguousarray(gb.astype(BF)[None, :]),
    }

    in_maps = []
    for c in range(NCORES):
        b = c // per_batch
        rows = rows_per_core[c]
        m = dict(wmaps)
        m["qf"] = np.ascontiguousarray(q[b][rows]).astype(BF)
        m["kc"] = kcs[b]
        m["vc"] = vcs[b]
        m["kval"] = kvals[b]
        m["qm"] = qmask[b][rows].astype(np.float32)[None, :]
        in_maps.append(m)
    return in_maps, rows_per_core, (QA, KC)


def kernel(_return_res=False, _run_kwargs=None, **inputs):
    run_kwargs = _run_kwargs or {}
    in_maps, rows_per_core, key = make_in_maps(inputs)
    if key not in _CACHE:
        _CACHE[key] = _build(*key)
    nc = _CACHE[key]
    res = run_bass_kernel_spmd(nc, in_maps, list(range(NCORES)), **run_kwargs)
    out = np.empty((B, Q, D), np.float32)
    per_batch = NCORES // B
    for c in range(NCORES):
        b = c // per_batch
        out[b, rows_per_core[c]] = res.results[c]["out"].astype(np.float32)
    if _return_res:
        return out, res
    return out


# revision 28
# speedup vs baseline: 1.2856x; 1.2856x over previous
"""Trainium2 Bass kernel for BaseAttnPredictNet (pre-LN MHA with zero-attn
slot, gated output combination, residual).

v2 strategy:
- Host compacts masked-out keys (mask=0 keys contribute exactly 0 to
  softmax) and deals query rows so each core gets ~equal unmasked rows,
  placed first in its 512-row block. Attention runs only on QA (~288)
  query columns and KC (~1152) compacted keys instead of 512x2176.
- Flipped PV: av[dh+den, q] accumulated in PSUM over key blocks with a
  valid-indicator column producing the softmax denominator; normalize by
  rank-1 PE broadcast of qm/den.
- Natural-layout gate / output projection / combine: activations serve as
  matmul lhsT so no transposes back from feature-major space are needed;
  gate bias enters via a ones-row rank-1 matmul.
- QK head pairs run as row-tiled (base partition 0/64) matmul pairs.
- Host pre-casts weights and k/v to bf16 (halves DMA).
- Sharding: cores 0-3 batch 0, cores 4-7 batch 1; host inverse-permutes
  row order on gather.
"""

import numpy as np
import ml_dtypes

import concourse.bass as bass
import concourse.bacc as bacc
import concourse.mybir as mybir
import concourse.tile as tile
from concourse.bass_utils import run_bass_kernel_spmd
from concourse.masks import make_identity

B, Q, KLEN, D = 2, 2048, 2048, 512
H, DH = 8, 64
P = 128
QS = 512          # query rows per core (output responsibility)
ND = D // P       # 4 feature blocks
NG = 2 * D // P
NCORES = 8
SCALE = 0.125
LN_EPS = 1e-5

F32 = mybir.dt.float32
BF16 = mybir.dt.bfloat16
AF = mybir.ActivationFunctionType
OP = mybir.AluOpType
BF = ml_dtypes.bfloat16


def _build(QA: int, KC: int) -> bass.Bass:
    NKC = KC // P
    nc = bacc.Bacc("TRN2", target_bir_lowering=False, debug=False)

    din = {}
    for name, shape, dt in (
        ("qf", [QS, D], BF16),
        ("kc", [KC, D], BF16),
        ("vc", [KC, D], BF16),
        ("wq", [D, D], BF16),
        ("wk", [D, D], BF16),
        ("wv", [D, D], BF16),
        ("wo", [D, D], BF16),
        ("wop", [D, D], BF16),
        ("gwq", [D, D], BF16),
        ("gbn", [1, D], BF16),
        ("kval", [P, NKC], F32),
        ("qm", [1, QS], F32),
    ):
        din[name] = nc.dram_tensor(name, shape, dt, kind="ExternalInput")
    out_d = nc.dram_tensor("out", [QS, D], BF16, kind="ExternalOutput")

    with tile.TileContext(nc) as tc:
        _body(nc, tc, din, out_d, QA, KC)
    nc.compile()
    return nc


def _body(nc, tc, din, out_d, QA, KC):
    NKC = KC // P
    from contextlib import ExitStack

    ctx = ExitStack()
    with ctx:
        persist = ctx.enter_context(tc.tile_pool(name="persist", bufs=1))
        stats = ctx.enter_context(tc.tile_pool(name="stats", bufs=6))

        ident_bf = persist.tile([P, P], BF16)
        make_identity(nc, ident_bf)
        ones_bf = persist.tile([P, P], BF16)
        nc.vector.memset(ones_bf, 1.0)
        eps_t = persist.tile([P, 1], F32)
        nc.vector.memset(eps_t, LN_EPS)

        kval = persist.tile([P, NKC], F32)
        gbn = persist.tile([1, D], BF16)
        qm_bc = persist.tile([P, QS], F32)

        # ---- persistent activations ----
        knT = persist.tile([P, ND, KC], BF16)
        vh_aug = persist.tile([P, NKC, H, DH + 1], BF16)
        qnT = persist.tile([P, ND, QA], BF16)
        qT = persist.tile([P, ND, QS], BF16)
        qhT = [persist.tile([P, QA], BF16, name=f"qhT{a}") for a in range(ND)]
        khT = [persist.tile([P, KC], BF16, name=f"khT{a}") for a in range(ND)]
        avT = persist.tile([P, ND, QA], BF16)
        po = persist.tile([P, ND, D], BF16)
        g_nat = persist.tile([P, ND, D], BF16)
        outn = persist.tile([P, ND, D], BF16)
        qf_s = persist.tile([P, ND, D], BF16)

        # PSUM budget is 8 banks; transpose pool (2) + projection pool (4)
        # coexist, both released (LIFO) before the attention pools (4+3+1).
        ptp = tc.alloc_tile_pool(name="ptp", bufs=2, space="PSUM")
        pp = tc.alloc_tile_pool(name="pp", bufs=4, space="PSUM")

        # natural-layout staging for k/v (released after use)
        vcp = tc.alloc_tile_pool(name="vcp", bufs=1)
        vc_s = vcp.tile([P, NKC, D], BF16)
        kcp = tc.alloc_tile_pool(name="kcp", bufs=1)
        kc_s = kcp.tile([P, NKC, D], BF16)

        # chunked input DMAs (3 blocks each => ~384KB)
        for c0 in range(0, NKC, 3):
            cw = min(3, NKC - c0)
            src = din["kc"][c0 * P : (c0 + cw) * P, :].rearrange("(c p) d -> p c d", p=P)
            nc.sync.dma_start(out=kc_s[:, c0 : c0 + cw, :], in_=src)
        for c0 in range(0, NKC, 3):
            cw = min(3, NKC - c0)
            src = din["vc"][c0 * P : (c0 + cw) * P, :].rearrange("(c p) d -> p c d", p=P)
            nc.sync.dma_start(out=vc_s[:, c0 : c0 + cw, :], in_=src)
        nc.sync.dma_start(
            out=qf_s, in_=din["qf"][:, :].rearrange("(a p) d -> p a d", p=P)
        )

        nc.sync.dma_start(out=kval, in_=din["kval"][:, :])
        nc.sync.dma_start(out=gbn, in_=din["gbn"][:, :])
        _qm_ap = din["qm"][:, :]
        nc.sync.dma_start(
            out=qm_bc,
            in_=bass.AP(tensor=_qm_ap.tensor, offset=_qm_ap.offset, ap=[[0, P], [1, QS]]),
        )

        # weights after activations: k/v/q feed the LN pipeline first
        w_s = {}
        for wname in ("wk", "wv", "wq", "wo", "wop", "gwq"):
            wt = persist.tile([P, ND, D], BF16, name=f"{wname}_s")
            nc.sync.dma_start(
                out=wt, in_=din[wname][:, :].rearrange("(b p) d -> p b d", p=P)
            )
            w_s[wname] = wt

        def ln_stats2(xtiles):
            """LN (bias,scale) for up to 2 [rows,D] tiles; batches the sqrt."""
            cw = len(xtiles)
            mv = stats.tile([P, 2, 2], F32, name="bnagg")
            for cc, (xt, rows) in enumerate(xtiles):
                st = stats.tile([P, 6], F32, name="bnst")
                nc.vector.bn_stats(out=st[:rows], in_=xt)
                nc.vector.bn_aggr(out=mv[:rows, cc, :], in_=st[:rows])
            std = stats.tile([P, 2], F32, name="std")
            nc.scalar.activation(
                out=std[:, :cw], in_=mv[:, 0:cw, 1], func=AF.Sqrt, bias=eps_t
            )
            rstd = stats.tile([P, 2], F32, name="rstd")
            nc.vector.reciprocal(rstd[:, :cw], std[:, :cw])
            nm2 = stats.tile([P, 2], F32, name="nm2")
            nc.vector.tensor_tensor(
                out=nm2[:, :cw], in0=mv[:, 0:cw, 0], in1=rstd[:, :cw], op=OP.mult
            )
            nc.vector.tensor_scalar_mul(nm2[:, :cw], nm2[:, :cw], -1.0)
            return nm2, rstd

        def ln_transpose(src_tile, nrows, dstT, apply_eng, copy_engs):
            """Per-128-row-block LN + PE transpose into dstT [P, ND, nrows]."""
            nblk = (nrows + P - 1) // P
            ei = 0
            for c0 in range(0, nblk, 2):
                cw = min(2, nblk - c0)
                xt = []
                for cc in range(cw):
                    rows = min(P, nrows - (c0 + cc) * P)
                    xt.append((src_tile[:rows, c0 + cc, :], rows))
                nm2, rstd = ln_stats2(xt)
                for cc in range(cw):
                    c = c0 + cc
                    rows = xt[cc][1]
                    xn = stats.tile([P, D], BF16, name="xnorm")
                    if apply_eng == "gps":
                        nc.gpsimd.tensor_scalar(
                            out=xn[:rows],
                            in0=xt[cc][0],
                            scalar1=nm2[:rows, cc : cc + 1],
                            scalar2=rstd[:rows, cc : cc + 1],
                            op0=OP.add,
                            op1=OP.mult,
                        )
                    else:
                        nc.vector.tensor_scalar(
                            out=xn[:rows],
                            in0=xt[cc][0],
                            scalar1=nm2[:rows, cc : cc + 1],
                            scalar2=rstd[:rows, cc : cc + 1],
                            op0=OP.add,
                            op1=OP.mult,
                        )
                    pt = ptp.tile([P, ND, P], BF16, name="pt")
                    for b in range(ND):
                        nc.tensor.transpose(
                            pt[:, b, :rows],
                            xn[:rows, b * P : (b + 1) * P],
                            ident_bf[:rows, :rows],
                        )
                    eng = copy_engs[ei % len(copy_engs)]
                    ei += 1
                    if eng == "act":
                        nc.scalar.copy(dstT[:, :, c * P : c * P + rows], pt[:, :, :rows])
                    elif eng == "gps":
                        nc.gpsimd.tensor_copy(dstT[:, :, c * P : c * P + rows], pt[:, :, :rows])
                    else:
                        nc.vector.tensor_copy(dstT[:, :, c * P : c * P + rows], pt[:, :, :rows])

        # ---- k: LN + transpose, then k projection ----
        ln_transpose(kc_s, KC, knT, "vec", ("act", "vec"))

        # khT[a] = Wk'.T @ knT   (j in chunks of KC/3)
        JC = KC // 3
        for a in range(ND):
            for j0 in range(0, KC, JC):
                ps = pp.tile([P, 512], F32, name="pp_t")
                for b in range(ND):
                    nc.tensor.matmul(
                        ps[:, :JC],
                        w_s["wk"][:, b, a * P : (a + 1) * P],
                        knT[:, b, j0 : j0 + JC],
                        start=(b == 0),
                        stop=(b == ND - 1),
                    )
                nc.scalar.copy(khT[a][:, j0 : j0 + JC], ps[:, :JC])
        kcp.release()

        # ---- v: LN + transpose (into temp), then vh (natural, valid-scaled) ----
        vtp = tc.alloc_tile_pool(name="vtp", bufs=1)
        vnT = vtp.tile([P, ND, KC], BF16)
        ln_transpose(vc_s, KC, vnT, "vec", ("vec", "act"))
        for c in range(NKC):
            ps = pp.tile([P, 512], F32, name="pp_t")
            for b in range(ND):
                nc.tensor.matmul(
                    ps,
                    vnT[:, b, c * P : (c + 1) * P],
                    w_s["wv"][:, b, :],
                    start=(b == 0),
                    stop=(b == ND - 1),
                )
            pp3 = ps.rearrange("p (h e) -> p h e", h=H)
            nc.scalar.activation(
                out=vh_aug[:, c, :, 0:DH],
                in_=pp3,
                func=AF.Copy,
                scale=kval[:, c : c + 1],
            )
            nc.vector.tensor_copy(
                vh_aug[:, c, :, DH : DH + 1],
                kval[:, c : c + 1].unsqueeze(1).broadcast_to((P, H, 1)),
            )
        vtp.release()
        vcp.release()

        # ---- q: LN + transpose (first QA rows), raw bf16 transpose (all rows) ----
        ln_transpose(qf_s, QA, qnT, "vec", ("vec", "act"))
        for a in range(ND):
            pt = ptp.tile([P, ND, P], BF16, name="pt")
            for b in range(ND):
                nc.tensor.transpose(pt[:, b, :], qf_s[:, a, b * P : (b + 1) * P], ident_bf)
            nc.vector.tensor_copy(qT[:, :, a * P : (a + 1) * P], pt)

        # qhT[a] = Wq'.T @ qnT
        for a in range(ND):
            ps = pp.tile([P, 512], F32, name="pp_t")
            for b in range(ND):
                nc.tensor.matmul(
                    ps[:, :QA],
                    w_s["wq"][:, b, a * P : (a + 1) * P],
                    qnT[:, b, :],
                    start=(b == 0),
                    stop=(b == ND - 1),
                )
            nc.vector.tensor_copy(qhT[a], ps[:, :QA])

        # ---- attention: per head-pair, QK row-tiled pairs -> exp -> PV ----
        pp.release()
        ptp.release()
        expp = tc.alloc_tile_pool(name="expp", bufs=3)
        psS = tc.alloc_tile_pool(name="psS", bufs=2, space="PSUM")
        pav = tc.alloc_tile_pool(name="pav", bufs=3, space="PSUM")
        pfb = tc.alloc_tile_pool(name="pfb", bufs=1, space="PSUM")

        for nb in range(ND):
            av2 = [pav.tile([P, 512], F32, name="pav_t") for _ in range(2)]
            for c in range(NKC):
                ps2 = psS.tile([P, 2, 512], F32, name="pS2")
                for i in range(2):
                    r0 = i * DH
                    nc.tensor.matmul(
                        ps2[:, i, :QA],
                        khT[nb][r0 : r0 + DH, c * P : (c + 1) * P],
                        qhT[nb][r0 : r0 + DH, :],
                        start=True,
                        stop=True,
                    )
                e2 = expp.tile([P, 2, QA], BF16, name="expS")
                nc.scalar.activation(
                    out=e2, in_=ps2[:, :, :QA], func=AF.Exp, scale=SCALE
                )
                for i in range(2):
                    nc.tensor.matmul(
                        av2[i][0 : DH + 1, :QA],
                        vh_aug[:, c, 2 * nb + i, :],
                        e2[:, i, :],
                        start=(c == 0),
                        stop=(c == NKC - 1),
                    )
            # normalize: avT rows = [head even 0:64, head odd 64:128].
            # Copy av to SBUF first so the PSUM bank frees for the next pair;
            # the slow reciprocal then runs off the critical path.
            for i in range(2):
                av = av2[i]
                avs = stats.tile([P, QA], BF16, name="avs")
                nc.vector.tensor_copy(avs[0 : DH + 1, :], av[0 : DH + 1, :QA])
                ftf = stats.tile([P, QA], F32, name="ftf")
                nc.vector.reciprocal(ftf[DH : DH + 1, :], avs[DH : DH + 1, :])
                ft = stats.tile([P, QA], BF16, name="ft")
                nc.vector.tensor_tensor(
                    out=ft[DH : DH + 1, :],
                    in0=ftf[DH : DH + 1, :],
                    in1=qm_bc[DH : DH + 1, :QA],
                    op=OP.mult,
                )
                fb = pfb.tile([P, 512], F32, name="fb_t")
                nc.tensor.matmul(
                    fb[0:DH, :QA],
                    ones_bf[DH : DH + 1, 0:DH],
                    ft[DH : DH + 1, :],
                    start=True,
                    stop=True,
                )
                fbs = stats.tile([P, QA], BF16, name="fbs")
                nc.vector.tensor_copy(fbs[0:DH, :], fb[0:DH, :QA])
                if i == 0:
                    nc.vector.tensor_tensor(
                        out=avT[0:DH, nb, :],
                        in0=avs[0:DH, :],
                        in1=fbs[0:DH, :],
                        op=OP.mult,
                    )
                else:
                    avtmp = stats.tile([P, QA], BF16, name="avtmp")
                    nc.vector.tensor_tensor(
                        out=avtmp[0:DH, :],
                        in0=avs[0:DH, :],
                        in1=fbs[0:DH, :],
                        op=OP.mult,
                    )
                    sh = pfb.tile([P, 512], F32, name="fb_t")
                    nc.tensor.matmul(
                        sh[DH : 2 * DH, :QA],
                        ident_bf[0:DH, 0:DH],
                        avtmp[0:DH, :],
                        start=True,
                        stop=True,
                    )
                    nc.vector.tensor_copy(avT[DH:P, nb, :], sh[DH : 2 * DH, :QA])

        # ---- output projection (natural): po[qblk] = avT.T @ Wo ----
        pfb.release()
        pav.release()
        psS.release()
        expp.release()
        pog = tc.alloc_tile_pool(name="pog", bufs=4, space="PSUM")
        NQB = (QA + P - 1) // P  # q blocks overlapping attention rows
        nc.gpsimd.memset(po[:, :, :], 0.0)
        for a in range(NQB):
            rows = min(P, QA - a * P)
            ps = pog.tile([P, 512], F32, name="pog_t")
            for b in range(ND):
                nc.tensor.matmul(
                    ps[:rows, :],
                    avT[:, b, a * P : a * P + rows],
                    w_s["wo"][:, b, :],
                    start=(b == 0),
                    stop=(b == ND - 1),
                )
            nc.vector.tensor_copy(po[:rows, a, :], ps[:rows, :])

        # ---- gate (natural): g = sigmoid(q@gwq + av_n@wop + gb) ----
        for a in range(ND):
            ps = pog.tile([P, 512], F32, name="pog_t")
            nc.tensor.matmul(
                ps, ones_bf[0:1, 0:P], gbn[0:1, :], start=True, stop=False
            )
            rows = min(max(QA - a * P, 0), P)
            for b in range(ND):
                nc.tensor.matmul(
                    ps,
                    qT[:, b, a * P : (a + 1) * P],
                    w_s["gwq"][:, b, :],
                    start=False,
                    stop=(b == ND - 1 and rows == 0),
                )
            if rows > 0:
                for b in range(ND):
                    nc.tensor.matmul(
                        ps[:rows, :],
                        avT[:, b, a * P : a * P + rows],
                        w_s["wop"][:, b, :],
                        start=False,
                        stop=(b == ND - 1),
                    )
            nc.scalar.activation(out=g_nat[:, a, :], in_=ps, func=AF.Sigmoid)

        # ---- combine: out = (q + po) + g*(q - po) ----
        for a in range(ND):
            s = stats.tile([P, D], BF16, name="fin_s")
            nc.vector.tensor_tensor(
                out=s, in0=qf_s[:, a, :], in1=po[:, a, :], op=OP.subtract
            )
            m = stats.tile([P, D], BF16, name="fin_m")
            nc.vector.tensor_tensor(out=m, in0=g_nat[:, a, :], in1=s, op=OP.mult)
            r = stats.tile([P, D], BF16, name="fin_r")
            nc.vector.tensor_tensor(
                out=r, in0=qf_s[:, a, :], in1=po[:, a, :], op=OP.add
            )
            nc.vector.tensor_tensor(out=outn[:, a, :], in0=m, in1=r, op=OP.add)

        dst = out_d[:, :].rearrange("(a p) d -> p a d", p=P)
        nc.sync.dma_start(out=dst, in_=outn)
        pog.release()


_CACHE: dict = {}


def _ceil(x, g):
    return -(-x // g) * g


def make_in_maps(inputs):
    q = np.asarray(inputs["query"], np.float32)
    k = np.asarray(inputs["key"], np.float32)
    v = np.asarray(inputs["value"], np.float32)
    wq = np.asarray(inputs["weight_q"], np.float32)
    wk = np.asarray(inputs["weight_k"], np.float32)
    wv = np.asarray(inputs["weight_v"], np.float32)
    wo = np.asarray(inputs["weight_o"], np.float32)
    gw = np.asarray(inputs["g_w"], np.float32)
    gb = np.asarray(inputs["g_b"], np.float32)
    qmask = np.asarray(inputs["query_mask"])
    kmask = np.asarray(inputs["key_mask"])
    gams = [np.asarray(inputs[n], np.float32) for n in ("q_gamma", "k_gamma", "v_gamma")]
    bets = [np.asarray(inputs[n], np.float32) for n in ("q_beta", "k_beta", "v_beta")]
    assert all(np.all(bt == 0.0) for bt in bets), "beta path not implemented"

    # fold LN gamma into projection weight rows (free on host)
    wqf = wq * gams[0][:, None]
    wkf = wk * gams[1][:, None]
    wvf = wv * gams[2][:, None]
    wopf = wo @ gw[D:, :]
    gwqf = gw[:D, :]

    per_batch = NCORES // B

    # --- key compaction (+ zero-attn slot) ---
    kidx = [np.where(kmask[b] != 0)[0] for b in range(B)]
    KC = _ceil(max(len(ix) + 1 for ix in kidx), P)
    NKC = KC // P
    kcs, vcs, kvals = [], [], []
    for b in range(B):
        ix = kidx[b]
        n = len(ix)
        kc = np.zeros((KC, D), np.float32)
        vc = np.zeros((KC, D), np.float32)
        kc[:n] = k[b][ix]
        vc[:n] = v[b][ix]
        kvc = np.zeros(KC, np.float32)
        kvc[: n + 1] = 1.0  # compacted keys + zero-attn slot
        kcs.append(kc.astype(BF))
        vcs.append(vc.astype(BF))
        kvals.append(np.ascontiguousarray(kvc.reshape(NKC, P).T))

    # --- query deal: each core gets its batch's unmasked rows c::4 first ---
    rows_per_core = []
    na_per_core = []
    for b in range(B):
        un = np.where(qmask[b] != 0)[0]
        ma = np.where(qmask[b] == 0)[0]
        parts = [list(un[c::per_batch]) for c in range(per_batch)]
        mi = 0
        for c in range(per_batch):
            need = QS - len(parts[c])
            parts[c] = parts[c] + list(ma[mi : mi + need])
            mi += need
        assert mi == len(ma)
        for c in range(per_batch):
            rows_per_core.append(np.array(parts[c], np.int64))
            na_per_core.append(int((qmask[b][parts[c]] != 0).sum()))
    QA = max(_ceil(max(na_per_core), 32), 32)

    wmaps = {
        "wq": np.ascontiguousarray(wqf.astype(BF)),
        "wk": np.ascontiguousarray(wkf.astype(BF)),
        "wv": np.ascontiguousarray(wvf.astype(BF)),
        "wo": np.ascontiguousarray(wo.astype(BF)),
        "wop": np.ascontiguousarray(wopf.astype(BF)),
        "gwq": np.ascontiguousarray(gwqf.astype(BF)),
        "gbn": np.ascontiguousarray(gb.astype(BF)[None, :]),
    }

    in_maps = []
    for c in range(NCORES):
        b = c // per_batch
        rows = rows_per_core[c]
        m = dict(wmaps)
        m["qf"] = np.ascontiguousarray(q[b][rows]).astype(BF)
        m["kc"] = kcs[b]
        m["vc"] = vcs[b]
        m["kval"] = kvals[b]
        m["qm"] = qmask[b][rows].astype(np.float32)[None, :]
        in_maps.append(m)
    return in_maps, rows_per_core, (QA, KC)


def kernel(_return_res=False, _run_kwargs=None, **inputs):
    run_kwargs = _run_kwargs or {}
    in_maps, rows_per_core, key = make_in_maps(inputs)
    if key not in _CACHE:
        _CACHE[key] = _build(*key)
    nc = _CACHE[key]
    res = run_bass_kernel_spmd(nc, in_maps, list(range(NCORES)), **run_kwargs)
    out = np.empty((B, Q, D), np.float32)
    per_batch = NCORES // B
    for c in range(NCORES):
        b = c // per_batch
        out[b, rows_per_core[c]] = res.results[c]["out"].astype(np.float32)
    if _return_res:
        return out, res
    return out
